# revision 32
# baseline (speedup 1.0000x reference)
"""JumpAttention (channel attention, cross-swapped values) on 8 trn2 cores.

Math (per batch b, per head h, hd=64):
  q,k,v = heads(x @ Wq), heads(x @ Wk), heads(x @ Wv)   laid out [hd, N]
  G_h   = q_h k_h^T (contraction over N)  ==  Wq_h^T (x^T x) Wk_h
  attn  = softmax(G / (||q|| ||k||) * temp, axis=d)
  y1+y2 = x2 @ F1 + x1 @ F2,  F_s = concat_h(Wv_h @ attn_s_h^T)

Kernel structure (v4):
  pass 1: S_s = x_s^T x_s from an fp8(e4m3) copy of x using DoubleRow
          matmuls (2 tokens per partition -> 2x fp16 PE rate). S is
          symmetric: the lower-left 128-block is never computed; the
          interlude reads it through the transposed upper-right block.
          Stream-0's ENTIRE interlude chain is emitted interleaved with
          stream-1's S matmuls so only stream-1's chain is on the
          critical path.
  interlude: T=S@W, G=Wq^T T, norms via ones-matmul of W*T, softmax,
          F_s = Wv @ attn_s^T.  ACT runs only Sqrt/Exp (+warmup loads);
          all copies go to DVE/Pool so the single sqrt->exp table
          switch hides under DVE work.
  pass 2: out = x2 @ F1 + x1 @ F2 with lhsT streamed from a HOST-
          TRANSPOSED fp16 x^T in DRAM (no PE transposes), fp16 output
          (host upcasts to f32).

  DMA traffic/core: 8MB (x fp8) + 16MB (x^T fp16) + 8MB (out) + 0.5MB
  consts, all on one in-order SP queue ordered x8 -> xt -> out so the
  DMA engines (the bottleneck at 360GB/s) never idle.

Sharding: pure data-parallel over B (B=8 == n_cores). No collectives.
"""

import os
import sys
from contextlib import ExitStack

import numpy as np
import ml_dtypes

for _p in ("/opt/trn_rl_repo",):
    if _p not in sys.path and os.path.isdir(_p):
        sys.path.insert(0, _p)

import concourse.bass as bass  # noqa: E402
import concourse.tile as tile  # noqa: E402
from concourse import bacc, mybir  # noqa: E402
from concourse.bass_utils import run_bass_kernel_spmd  # noqa: E402

B, N_FULL, C = 8, 16384, 256
H, HD = 4, 64
NCORES = 8

F32 = mybir.dt.float32
F16 = mybir.dt.float16
F8 = mybir.dt.float8e4
AF = mybir.ActivationFunctionType
DR = mybir.MatmulPerfMode.DoubleRow

# x8 slab sizes per stream (tokens): small first slab primes PE early,
# small last slab shortens the S-completion tail after the x8 stream ends.
SLAB_SIZES = [1024, 4096, 4096, 4096, 2048, 1024]
SLAB_MAX = max(SLAB_SIZES)
X8_BUFS = 5
WINT = 2048  # tokens per xt window (4 x 512KB fp16 tiles) == out slab
XT_BUFS = 6  # xt windows in flight per (stream, chunk)
BLOBW = 2176  # wkq pairs [0:1024], identity [1024:1152], wvt [1152:2176]


def _build(n_tokens: int):
    nc = bacc.Bacc(
        "TRN2", target_bir_lowering=False, debug=False, num_devices=NCORES
    )
    x8 = [
        nc.dram_tensor(f"x{s + 1}f8", [n_tokens, C], F8, kind="ExternalInput").ap()
        for s in range(2)
    ]
    xt = [
        nc.dram_tensor(f"xt{s + 1}", [C, n_tokens], F16, kind="ExternalInput").ap()
        for s in range(2)
    ]
    wblob = nc.dram_tensor("wblob", [128, BLOBW], F16, kind="ExternalInput").ap()
    tmpd = nc.dram_tensor("tmpd", [128, 4], F32, kind="ExternalInput").ap()
    out = nc.dram_tensor("out", [n_tokens, C], F16, kind="ExternalOutput").ap()

    with tile.TileContext(nc) as tc, ExitStack() as ctx:
        _kernel(ctx, tc, out, x8, xt, wblob, tmpd, n_tokens)
    nc.compile()
    return nc


def _kernel(ctx, tc, out, x8in, xtin, wblob, tmpd, nt):
    nc = tc.nc
    nwin = nt // WINT
    assert sum(SLAB_SIZES) == nt
    singles = ctx.enter_context(tc.tile_pool(name="singles", bufs=1))

    # ---- constants / weights ----
    # blob cols: [0:1024] wkq pairs ([Wk|Wq] per 128-row chunk),
    # [1024:1152] identity, [1152:2176] wvt heads (rows 0-63).
    blob_sb = singles.tile([128, BLOBW], F16, tag="blob", name="blob")
    wkq_sb = [blob_sb[:, r * 512 : (r + 1) * 512] for r in range(2)]
    wq_sb = [blob_sb[:, r * 512 + C : (r + 1) * 512] for r in range(2)]
    wk_sb = [blob_sb[:, r * 512 : r * 512 + C] for r in range(2)]
    ident_sb = blob_sb[:, 1024:1152]
    wvt_sb = [blob_sb[0:HD, 1152 + h * C : 1152 + (h + 1) * C] for h in range(H)]
    tmps_sb = singles.tile([128, 4], F32, tag="tmps", name="tmps")
    tmp_sb = [tmps_sb[:, 2 * s : 2 * s + 2] for s in range(2)]
    ones_col = singles.tile([128, 1], F16, tag="ones_col", name="ones_col")
    nc.vector.memset(ones_col[:], 1.0)
    ones_row = singles.tile([1, 128], F16, tag="ones_row", name="ones_row")
    nc.vector.memset(ones_row[:], 1.0)
    warm = singles.tile([1, 1], F32, tag="warm", name="warm")
    nc.vector.memset(warm[:], 1.0)
    # load the sqrt act table at t~0, off the critical path (every set
    # contains Copy, so only Sqrt/Exp ever force a table switch)
    nc.scalar.activation(warm[:], warm[:], AF.Sqrt)

    S_sb = [
        [singles.tile([128, C], F16, tag=f"ssb{s}{c}", name=f"ssb{s}{c}") for c in range(2)]
        for s in range(2)
    ]
    F_sb = [
        [singles.tile([128, C], F16, tag=f"f{s}{jc}", name=f"f{s}{jc}") for jc in range(2)]
        for s in range(2)
    ]
    # stream-0 copies of G / bc parked in SBUF so their PSUM frees early
    g_sb0 = [singles.tile([128, C], F32, tag=f"gsb{cc}", name=f"gsb{cc}") for cc in range(2)]
    bc_sb0 = singles.tile([128, C], F32, tag="bcsb", name="bcsb")

    # ---- DMA issue plan (single in-order SP queue == transfer order):
    # x8 slab0, [slabs 1..], consts mid-stream, remaining x8, xt windows,
    # out slabs last.
    x8_pool = ctx.enter_context(tc.tile_pool(name="x8", bufs=X8_BUFS))
    nslab = len(SLAB_SIZES)
    nslabs_total = 2 * nslab

    def issue_x8(k):
        s, si = divmod(k, nslab)
        sz = SLAB_SIZES[si]
        base = sum(SLAB_SIZES[:si])
        sl = x8_pool.tile([128, SLAB_MAX * C // 128], F8, tag="x8slab", name="x8slab")
        nc.sync.dma_start(
            out=sl[:, 0 : sz * C // 128].rearrange(
                "p (q c) -> p q c", q=sz // 128
            ),
            in_=x8in[s][base : base + sz, :].rearrange("(p q) c -> p q c", p=128),
        )
        return sl

    # wkq+ident+temps first (needed by the overlapped stream-0 interlude
    # from ~16us); the wvt half of the blob is only needed by stage F
    # (~30us) and transfers after the last x8 slab.
    nc.sync.dma_start(out=blob_sb[:, 0:1152], in_=wblob[:, 0:1152])
    nc.sync.dma_start(out=tmps_sb[:], in_=tmpd[:, :])
    x8_tiles = [issue_x8(k) for k in range(X8_BUFS)]

    xt_pool = ctx.enter_context(tc.tile_pool(name="xt", bufs=XT_BUFS))
    xt_sb = {}

    def issue_xt_window(w):
        for s in range(2):
            for jc in range(2):
                xtile = xt_pool.tile(
                    [128, WINT], F16, tag=f"xt{s}{jc}", name=f"xt{s}{jc}"
                )
                nc.sync.dma_start(
                    out=xtile[:],
                    in_=xtin[s][jc * 128 : (jc + 1) * 128, w * WINT : (w + 1) * WINT],
                )
                xt_sb[s, jc, w] = xtile

    # ---- interlude stage emitters (per stream) ----
    # cross-engine copy helpers: s1's chain rides DVE (faster); s0's mostly
    # Pool so it never contends with anything on the critical path.
    def cp(eng, dst, src):
        if eng == "v":
            nc.vector.tensor_copy(dst, src)
        elif eng == "p":
            nc.gpsimd.tensor_copy(dst, src)
        else:
            nc.scalar.activation(dst, src, AF.Copy)

    il = {}

    def s_lhsT(s, jc, ic, lo, hi):
        # S[128:256, 0:128] == S[0:128, 128:256]^T (symmetric, matmul
        # transposes lhsT) — the lower-left block is never materialized.
        if jc == 1 and ic == 0:
            return S_sb[s][0][:, 128 + lo : 128 + hi]
        return S_sb[s][jc][:, ic * 128 + lo : ic * 128 + hi]

    def stage_copyS(s, S_ps):
        e0, e1 = ("p", "v") if s == 0 else ("v", "p")
        cp(e0, S_sb[s][0][:], S_ps[s][:, 0:256])
        cp(e1, S_sb[s][1][:, 128:256], S_ps[s][:, 256:384])

    def stage_A(s, big, sbp):
        # [T_k | T_q] = S @ [Wk | Wq]
        for ic in range(2):
            tp = big.tile([128, 2 * C], F32, tag="ilbig", name="ilbig")
            for jc in range(2):
                nc.tensor.matmul(
                    tp[:],
                    lhsT=s_lhsT(s, jc, ic, 0, 128),
                    rhs=wkq_sb[jc],
                    start=(jc == 0),
                    stop=(jc == 1),
                )
            st = sbp.tile([128, 2 * C], F16, tag=f"t{ic}_{s}", name=f"t{ic}_{s}")
            # k-half on DVE first so stage B's G matmuls unblock sooner;
            # q-half on Pool in parallel (only feeds the norm path)
            cp("v", st[:, 0:C], tp[:, 0:C])
            cp("p", st[:, C : 2 * C], tp[:, C : 2 * C])
            il["k", s, ic] = st[:, 0:C]
            il["q", s, ic] = st[:, C : 2 * C]

    def stage_B(s, big, sbp):
        # G = Wq^T @ T_k ; U = W * T
        for cc in range(2):
            g = big.tile([128, C], F32, tag="ilbig", name="ilbig")
            for ic in range(2):
                nc.tensor.matmul(
                    g[:],
                    lhsT=wq_sb[ic][:, cc * 128 : (cc + 1) * 128],
                    rhs=il["k", s, ic],
                    start=(ic == 0),
                    stop=(ic == 1),
                )
            if s == 0:
                cp("v" if cc == 0 else "p", g_sb0[cc][:], g[:])
                il["g", s, cc] = g_sb0[cc]
            else:
                il["g", s, cc] = g
        # all u-muls on DVE: Pool's multiply is ~3x slower and both the
        # q-norm and k-norm paths gate the critical chain
        for nm, ic in [("k", 0), ("k", 1), ("q", 0), ("q", 1)]:
            w_sb = wq_sb if nm == "q" else wk_sb
            u = sbp.tile([128, C], F16, tag=f"u{nm}{ic}_{s}", name=f"u{nm}{ic}_{s}")
            nc.vector.tensor_mul(u[:], w_sb[ic], il[nm, s, ic])
            il["u", nm, s, ic] = u

    def stage_C(s, big, small, sbp):
        # norms: ||q_c||^2 = sum_r Wq[r,c]*T_q[r,c] via ones-matmuls into
        # one packed psum bank, then invq = temp/sqrt(nq),
        # invk16 = 1/sqrt(nk) in fp16. (Packing several accumulation
        # groups per bank is HW-exact; only the interp's 2KB zero-region
        # model diverges, and values never run through the interp here.)
        nrm = small.tile([128, 384], F32, tag="ilsmall", name="ilsmall")
        for cc in range(2):
            for ic in range(2):
                nc.tensor.matmul(
                    nrm[:, cc : cc + 1],
                    lhsT=il["u", "q", s, ic][:, cc * 128 : (cc + 1) * 128],
                    rhs=ones_col[:],
                    start=(ic == 0),
                    stop=(ic == 1),
                    skip_group_check=True,
                )
        for ic in range(2):
            nc.tensor.matmul(
                nrm[0:1, 4 : 4 + C],
                lhsT=ones_col[:],
                rhs=il["u", "k", s, ic],
                start=(ic == 0),
                stop=(ic == 1),
                skip_group_check=True,
            )
        for cc in range(2):
            iq = sbp.tile([128, 1], F32, tag=f"invq{cc}_{s}", name=f"invq{cc}_{s}")
            nc.scalar.activation(iq[:], nrm[:, cc : cc + 1], AF.Sqrt)
            nc.vector.reciprocal(iq[:], iq[:])
            nc.vector.tensor_mul(iq[:], iq[:], tmp_sb[s][:, cc : cc + 1])
            il["invq", s, cc] = iq
        ik = sbp.tile([1, C], F32, tag=f"invk_{s}", name=f"invk_{s}")
        nc.scalar.activation(ik[:], nrm[0:1, 4 : 4 + C], AF.Sqrt)
        ikf = sbp.tile([1, C], F16, tag=f"invk16_{s}", name=f"invk16_{s}")
        with nc.allow_low_precision(reason="1/||k|| fits f16 comfortably"):
            nc.vector.reciprocal(ikf[:], ik[:])
        # broadcast invk down the partitions with a ones-matmul
        bc = big.tile([128, C], F32, tag="ilbig", name="ilbig")
        nc.tensor.matmul(bc[:], lhsT=ones_row[:], rhs=ikf[:], start=True, stop=True)
        if s == 0:
            cp("p", bc_sb0[:], bc[:])
            il["bc", s] = bc_sb0
        else:
            il["bc", s] = bc

    def stage_D(s, sbp):
        # logits = G * invq * bc_invk (per head block). No row-max
        # subtraction: q,k are unit vectors so |logit| <= |temp| by
        # Cauchy-Schwarz — exp() is overflow-safe directly, and softmax
        # is shift-invariant so the result is identical.
        for cc in range(2):
            eng = nc.vector if s == 1 else nc.gpsimd
            lp = sbp.tile([128, HD], F32, tag=f"lp{cc}_{s}", name=f"lp{cc}_{s}")
            for half in range(2):
                h = 2 * cc + half
                rs = slice(half * 64, (half + 1) * 64)
                cs = slice(h * 64, (h + 1) * 64)
                eng.tensor_scalar_mul(
                    lp[rs, :], il["g", s, cc][:][rs, cs], il["invq", s, cc][:][rs, :]
                )
                eng.tensor_mul(lp[rs, :], lp[rs, :], il["bc", s][:][rs, cs])
            il["lp", s, cc] = lp

    def stage_E(s, sbp):
        # attn = exp(lp) / rowsum  (fp16)
        for cc in range(2):
            pexp = sbp.tile([128, HD], F32, tag=f"pexp{cc}_{s}", name=f"pexp{cc}_{s}")
            sm = sbp.tile([128, 1], F32, tag=f"sm{cc}_{s}", name=f"sm{cc}_{s}")
            nc.scalar.activation(
                pexp[:], il["lp", s, cc][:], AF.Exp,
                accum_out=sm[:],
            )
            il["pexp", s, cc] = (pexp, sm)
        for cc in range(2):
            pexp, sm = il["pexp", s, cc]
            nc.vector.reciprocal(sm[:], sm[:])
            a16 = sbp.tile([128, HD], F16, tag=f"a16{cc}_{s}", name=f"a16{cc}_{s}")
            eng = nc.vector if cc == 0 else nc.gpsimd
            eng.tensor_scalar_mul(a16[:], pexp[:], sm[:])
            il["a16", s, cc] = a16

    def stage_F(s, big, small, sbp):
        # F_s = concat_h(Wv_h @ attn_s_h^T)
        for cc in range(2):
            atp = big.tile([HD, 128], F16, tag="ilbig", name="ilbig")
            nc.tensor.transpose(atp[:], il["a16", s, cc][:], ident_sb)
            at = sbp.tile([HD, 128], F16, tag=f"at{cc}_{s}", name=f"at{cc}_{s}")
            cp("v" if cc == 0 else "a", at[:], atp[:])
            il["at", s, cc] = at
        for jc in range(2):
            fp = big.tile([128, C], F32, tag="ilbig", name="ilbig")
            for h in range(H):
                cc, half = divmod(h, 2)
                nc.tensor.matmul(
                    fp[:, h * 64 : (h + 1) * 64],
                    lhsT=wvt_sb[h][:, jc * 128 : (jc + 1) * 128],
                    rhs=il["at", s, cc][:, half * 64 : (half + 1) * 64],
                    start=True,
                    stop=True,
                )
            cp("v" if jc == 0 else "a", F_sb[s][jc][:], fp[:])

    wsrc = singles.tile([128, 512], F16, tag="wsrc", name="wsrc")
    nc.vector.memset(wsrc[:], 0.0)

    # ================= pass 1 + overlapped stream-0 interlude =============
    # S row-chunk m0 covers rows [m0*64, m0*64+64); col range per chunk
    # (symmetry: rows 128+ only need cols 128:256).
    chunk_cols = [(0, 256), (0, 256), (128, 128), (128, 128)]

    # dummy matmuls injected where PE would otherwise catch up to the x8
    # stream and idle: an idle gap resets the tensor engine's p-state ramp
    # (costing ~1.5us of mid-clock each), so burning ~200ns keeping it
    # busy is a large net win.
    STALL_PLUGS = {1: 2, 2: 2, 3: 1, 4: 1, 5: 1}

    with ExitStack() as p1:
        psS = p1.enter_context(tc.tile_pool(name="psS", bufs=1, space="PSUM"))
        big = p1.enter_context(tc.tile_pool(name="ilbig", bufs=4, space="PSUM"))
        small = p1.enter_context(tc.tile_pool(name="ilsmall", bufs=1, space="PSUM"))
        psW = p1.enter_context(tc.tile_pool(name="psW", bufs=1, space="PSUM"))
        sbp = p1.enter_context(tc.tile_pool(name="ilsb", bufs=1))
        wp = psW.tile([1, 512], F32, tag="wp", name="wp")

        def plug(n):
            for _ in range(n):
                nc.tensor.matmul(
                    wp[:], lhsT=ones_col[:], rhs=wsrc[:], start=True,
                    stop=True, skip_group_check=True,
                )
        # one packed bank per stream: [:, 0:256] = S rows 0:128 (all cols),
        # [:, 256:384] = S rows 128:256, cols 128:256
        S_ps = [
            psS.tile([128, 384], F32, tag=f"s{s}", name=f"s{s}")
            for s in range(2)
        ]

        # actions interleaved after slab k's matmuls (slabs 0-5 = stream 0,
        # 6-11 = stream 1); consts DMA rides after stream-1's first slab.
        def post_slab(k):
            if k == 0:
                pass
            elif k == nslab - 1:
                stage_copyS(0, S_ps)
            elif k == nslab:
                # last x8 slab is queued by now; wvt rides behind it
                nc.sync.dma_start(
                    out=blob_sb[:, 1152:BLOBW], in_=wblob[:, 1152:BLOBW]
                )
            elif k == nslab + 1:
                stage_A(0, big, sbp)
            elif k == nslab + 2:
                stage_B(0, big, sbp)
            elif k == nslab + 3:
                stage_C(0, big, small, sbp)
            elif k == nslab + 4:
                stage_D(0, sbp)

        npair_total = nt // 256
        ti = 0
        for k in range(nslabs_total):
            s, si = divmod(k, nslab)
            sl = x8_tiles[k]
            for t in range(SLAB_SIZES[si] // 256):
                pv = sl[:, t * 2 * C : (t + 1) * 2 * C].rearrange(
                    "p (j c) -> p j c", j=2
                )
                for m0 in range(4):
                    c0, cw = chunk_cols[m0]
                    po = 0 if m0 < 2 else 256
                    nc.tensor.matmul(
                        S_ps[s][
                            (m0 % 2) * 64 : (m0 % 2) * 64 + 64, po : po + cw
                        ],
                        lhsT=pv[:, :, m0 * 64 : (m0 + 1) * 64],
                        rhs=pv[:, :, c0 : c0 + cw],
                        start=(ti % npair_total == 0),
                        stop=(ti % npair_total == npair_total - 1),
                        perf_mode=DR,
                        skip_group_check=True,
                    )
                ti += 1
            if k + X8_BUFS < nslabs_total:
                x8_tiles.append(issue_x8(k + X8_BUFS))
            plug(STALL_PLUGS.get(k, 0))
            post_slab(k)

        # xt windows 0..XT_BUFS-1 into fresh buffers (the rest are issued
        # inside the pass-2 loop once their buffer's readers exist)
        for w in range(min(XT_BUFS, nwin)):
            issue_xt_window(w)

        # ---- stream-1 critical chain (s0's E/F fill engine gaps) ----
        stage_copyS(1, S_ps)
        stage_A(1, big, sbp)
        stage_B(1, big, sbp)
        stage_C(1, big, small, sbp)
        stage_D(1, sbp)
        stage_E(0, sbp)  # first Exp: the sqrt->exp table switch hides here
        stage_E(1, sbp)
        stage_F(0, big, small, sbp)
        stage_F(1, big, small, sbp)

    # ================= pass 2: out = x2@F1 + x1@F2 =================
    with ExitStack() as p2:
        psO = p2.enter_context(tc.tile_pool(name="psO", bufs=6, space="PSUM"))
        opool = p2.enter_context(tc.tile_pool(name="opool", bufs=6))
        pairs = [(1, F_sb[0]), (0, F_sb[1])]  # y1 = x2@F1, y2 = x1@F2
        tpo = WINT // 128
        for w in range(nwin):
            # flush granule: whole window normally; half-window for the
            # final window so the last DMA trails the last matmul closely
            ngr = 2 if w == nwin - 1 else 1
            gt = tpo // ngr  # psum tiles per granule
            for g in range(ngr):
                # parity-split staging: DVE owns even psum tiles, ACT odd
                osl = [
                    opool.tile(
                        [128, gt // 2 * C], F16, tag=f"osl{par}", name=f"osl{par}"
                    )
                    for par in range(2)
                ]
                for tg in range(gt):
                    t = g * gt + tg
                    op = psO.tile([128, C], F32, tag="op", name="op")
                    idx = 0
                    for sx, fs in pairs:
                        for jc in range(2):
                            nc.tensor.matmul(
                                op[:],
                                lhsT=xt_sb[sx, jc, w][:, t * 128 : (t + 1) * 128],
                                rhs=fs[jc][:],
                                start=(idx == 0),
                                stop=(idx == 3),
                            )
                            idx += 1
                    dst = osl[tg % 2][:, tg // 2 * C : (tg // 2 + 1) * C]
                    if tg % 2 == 0:
                        nc.vector.tensor_copy(dst, op[:])
                    else:
                        nc.scalar.activation(dst, op[:], AF.Copy)
                base = w * WINT + g * gt * 128
                for par in range(2):
                    nc.sync.dma_start(
                        out=out[base : base + gt * 128, :].rearrange(
                            "(t two p) c -> two p t c", p=128, two=2
                        )[par],
                        in_=osl[par][:].rearrange("p (t c) -> p t c", t=gt // 2),
                    )
            if w + XT_BUFS < nwin:
                issue_xt_window(w + XT_BUFS)


def _host_prep(w_qkv, temperature, temperature2):
    w = np.asarray(w_qkv, dtype=np.float32)
    wq = w[:, 0:C].astype(np.float16)
    wk = w[:, C : 2 * C].astype(np.float16)
    wvt = np.ascontiguousarray(w[:, 2 * C : 3 * C].T.reshape(H, HD, C)).astype(
        np.float16
    )
    blob = np.zeros((128, BLOBW), dtype=np.float16)
    for r in range(2):
        blob[:, r * 512 : r * 512 + C] = wk[r * 128 : (r + 1) * 128, :]
        blob[:, r * 512 + C : (r + 1) * 512] = wq[r * 128 : (r + 1) * 128, :]
    blob[:, 1024:1152] = np.eye(128, dtype=np.float16)
    for h in range(H):
        blob[0:HD, 1152 + h * C : 1152 + (h + 1) * C] = wvt[h]
    tmp = []
    for tarr in (temperature, temperature2):
        t = np.asarray(tarr, dtype=np.float32).reshape(H)
        tmp.append(
            np.stack(
                [np.repeat(t[[0, 1]], 64), np.repeat(t[[2, 3]], 64)], axis=1
            ).astype(np.float32)
        )
    tmpd = np.concatenate(tmp, axis=1).astype(np.float32)
    return blob, tmpd


_NC_CACHE = {}
LAST_RESULT = None


def _get_nc(n_tokens):
    if n_tokens not in _NC_CACHE:
        _NC_CACHE[n_tokens] = _build(n_tokens)
    return _NC_CACHE[n_tokens]


def kernel(x1, x2, w_qkv, temperature, temperature2):
    global LAST_RESULT
    x1 = np.asarray(x1, dtype=np.float32)
    x2 = np.asarray(x2, dtype=np.float32)
    b, n, c = x1.shape
    assert c == C and b == NCORES, (b, n, c)
    wblob, tmpd = _host_prep(w_qkv, temperature, temperature2)
    nc = _get_nc(n)
    in_maps = []
    for i in range(NCORES):
        m = {"wblob": wblob, "tmpd": tmpd}
        for s, x in enumerate((x1, x2)):
            m[f"x{s + 1}f8"] = np.ascontiguousarray(x[i]).astype(
                ml_dtypes.float8_e4m3
            )
            m[f"xt{s + 1}"] = np.ascontiguousarray(x[i].T).astype(np.float16)
        in_maps.append(m)
    res = run_bass_kernel_spmd(nc, in_maps, list(range(NCORES)))
    LAST_RESULT = res
    return np.stack([r["out"].astype(np.float32) for r in res.results]).reshape(
        b, n, c
    )
